# revision 43
# baseline (speedup 1.0000x reference)
"""CQATokenLearner fused Trainium2 kernel (v3).

Pure data parallel over batch: B=32 split as 4 batches per NeuronCore x 8 cores.
Weights replicated. Everything fused on-chip.

v3 design (v1 = 367us fp32r, v2 = 311us bf16):
  * all big-path data in bf16; x shipped twice (token-major for pooling,
    feature-major for the MLP) in HOST-PERMUTED layouts so every DMA
    descriptor is a contiguous 4KB block (v2's 1KB descriptors made the
    x stream the kernel tail).
  * sel logits FEATURE-major ([n, t]) - 512-row matmuls with a small
    stationary (v2's token-major variant quadrupled PE instruction count).
    Softmax denominators fall out of the exp's accum_out for free.
  * b2 dropped entirely (softmax over T is invariant to per-n shifts).
  * deferred CQA: batch b's CQA (LN1 -> sim -> softmaxes -> A/M/B -> cat ->
    LN2) is emitted after phase A of batch b+1 and its ACT exps are anchored
    into batch b+1's exp table window. This removes the per-batch PE bubble
    where next-batch gelus waited on the long serial CQA chain.
  * A/M/B matmuls in fp32r (1 cyc/row at 512 free) instead of fp32 (4x).
  * PSUM->SBUF staging on DVE; ACT only does gelu/exp/LN/softmax.

Exactness tricks kept from v1: no max-subtraction for the T-softmax (logits
are O(1)); softmax denominator cancels in LN1 except through eps, handled by
the eps*denom^2 bias; rsqrt via integer-magic seed + 3 Newton steps (DVE only,
no ACT table).
"""

import sys

sys.path.insert(0, "/opt/trn_rl_repo")

import numpy as np
import ml_dtypes

import concourse.bacc as bacc
import concourse.bass as bass
import concourse.mybir as mybir
import concourse.tile as tile
import concourse.masks as masks
import concourse.bass_utils as bass_utils
from concourse.tile_rust import add_dep_helper

F32 = mybir.dt.float32
F32R = mybir.dt.float32r
BF16 = mybir.dt.bfloat16
I32 = mybir.dt.int32
AF = mybir.ActivationFunctionType
ALU = mybir.AluOpType
AX = mybir.AxisListType

B, T, L, D, N = 32, 4096, 32, 512, 64
NCORES = 8
BS = B // NCORES          # batches per core
MACRO = 512               # tokens per macro tile
NMACRO = T // MACRO       # 8
JT = MACRO // 128         # 4 token sub-tiles per macro
KC = D // 128             # 4 chunks of the feature dim
EXPCH = 2048              # tokens per exp-instruction chunk
MPE = EXPCH // MACRO      # macros per exp chunk (4)
EPS = 1e-5

TRACE = False             # set by test harness for profiling


def _r(ap):
    """fp32r view of an fp32 AP (full-rate PE matmul mode)."""
    return ap.bitcast(F32R)


def _layer_norm(nc, cq, small, src, g_bc, b_bc, out_tile, pfx, eps_ap,
                out_r=False):
    # eps_ap: [P,1] per-partition eps bias added to var (LN1 passes
    # eps*denom^2 so the skipped softmax normalization stays exact)
    """LN over the free axis (D). src may be PSUM or SBUF, [P, D]."""
    P = src.shape[0]
    s = small.tile([P, 1], F32, tag="ln_s")
    nc.vector.reduce_sum(s[:], src, axis=AX.X)
    negmu = small.tile([P, 1], F32, tag="ln_negmu")
    nc.vector.tensor_scalar_mul(negmu[:], s[:], -1.0 / D)
    cent = cq.tile([P, D], F32, tag="ln_cent")
    nc.scalar.activation(cent[:], src, AF.Identity, bias=negmu[:])
    sq = cq.tile([P, D], F32, tag="ln_sq")
    vs = small.tile([P, 1], F32, tag="ln_vs")
    nc.scalar.activation(sq[:], cent[:], AF.Square, accum_out=vs[:])
    # veps = var + eps; rsqrt via DVE-only magic-number seed + 3 Newton steps
    veps = small.tile([P, 1], F32, tag="ln_veps")
    nc.scalar.activation(veps[:], vs[:], AF.Identity, scale=1.0 / D, bias=eps_ap)
    ish = small.tile([P, 1], I32, tag="ln_ish")
    nc.vector.tensor_scalar(
        ish[:], veps[:].bitcast(I32), 1, None, op0=ALU.arith_shift_right
    )
    fsh = small.tile([P, 1], F32, tag="ln_fsh")
    nc.vector.tensor_copy(fsh[:], ish[:])
    nc.vector.tensor_scalar(
        fsh[:], fsh[:], -1.0, float(0x5F3759DF), op0=ALU.mult, op1=ALU.add
    )
    rbits = small.tile([P, 1], I32, tag="ln_rbits")
    nc.vector.tensor_copy(rbits[:], fsh[:])
    r = rbits[:].bitcast(F32)
    for it in range(2):
        t1 = small.tile([P, 1], F32, tag=f"ln_nt{it}")
        nc.vector.tensor_mul(t1[:], veps[:], r)
        nc.vector.tensor_mul(t1[:], t1[:], r)
        nc.vector.tensor_scalar(t1[:], t1[:], -0.5, 1.5, op0=ALU.mult, op1=ALU.add)
        rn = small.tile([P, 1], F32, tag=f"ln_rn{it}")
        nc.vector.tensor_mul(rn[:], r, t1[:])
        r = rn[:]
    tmp = cq.tile([P, D], F32, tag="ln_tmp")
    nc.vector.scalar_tensor_tensor(
        tmp[:], cent[:], r, g_bc, op0=ALU.mult, op1=ALU.mult
    )
    dst = out_tile[:].bitcast(F32R) if out_r else out_tile[:]
    nc.vector.tensor_add(dst, tmp[:], b_bc)


def build_program():
    nc = bacc.Bacc(
        "TRN2",
        target_bir_lowering=False,
        debug=False,
        enable_asserts=False,
        num_devices=NCORES,
    )

    # host-permuted: x_d[b, m, p] = x[b, m*512 + j*128 + p, :] flat over (j, d)
    x_d = nc.dram_tensor(
        "x", [BS, NMACRO, 128, JT * D], BF16, kind="ExternalInput"
    ).ap()
    # host-permuted: xt_d[b, m, p] = xT[b, c*128 + p, m*512:(m+1)*512] flat (c, t)
    xt_d = nc.dram_tensor(
        "xt", [BS, NMACRO, 128, KC * MACRO], BF16, kind="ExternalInput"
    ).ap()
    q_d = nc.dram_tensor("q", [BS, L, D], F32R, kind="ExternalInput").ap()
    # host-transposed q: qt_d[b, p] = q[b, :, c*128+p].T flat over (c, l)
    qt_d = nc.dram_tensor("qt", [BS, 128, KC * L], F32, kind="ExternalInput").ap()
    # host-permuted: w1_d[p] = w1[c*128 + p, :] flat over (c, d)
    w1_d = nc.dram_tensor("w1", [128, KC * D], BF16, kind="ExternalInput").ap()
    b1_d = nc.dram_tensor("b1", [D], F32, kind="ExternalInput").ap()
    w2_d = nc.dram_tensor("w2", [128, KC * N], BF16, kind="ExternalInput").ap()
    g1_d = nc.dram_tensor("g1", [D], F32, kind="ExternalInput").ap()
    be1_d = nc.dram_tensor("be1", [D], F32, kind="ExternalInput").ap()
    wf_d = nc.dram_tensor(
        "wf", [128, 4, KC * D], BF16, kind="ExternalInput"
    ).ap()
    bfsum_d = nc.dram_tensor("bfsum", [D], F32, kind="ExternalInput").ap()
    g2_d = nc.dram_tensor("g2", [D], F32, kind="ExternalInput").ap()
    be2_d = nc.dram_tensor("be2", [D], F32, kind="ExternalInput").ap()
    out_d = nc.dram_tensor("out", [BS, N, D], F32, kind="ExternalOutput").ap()

    with tile.TileContext(nc) as tc:
        with (
            tc.tile_pool(name="const", bufs=1) as constp,
            tc.tile_pool(name="xin", bufs=12) as xp,
            tc.tile_pool(name="xt_sb", bufs=6) as xts,
            tc.tile_pool(name="ht_sb", bufs=2) as hts,
            tc.tile_pool(name="qin", bufs=2) as qp,
            tc.tile_pool(name="ht_ps", bufs=3, space="PSUM") as htp,
            tc.tile_pool(name="sel_ps", bufs=1, space="PSUM") as sep,
            tc.tile_pool(name="feat_ps", bufs=2, space="PSUM") as fpp,
            tc.tile_pool(name="cq_ps", bufs=2, space="PSUM") as cqp,
            tc.tile_pool(name="seln", bufs=2) as selnp,
            tc.tile_pool(name="et", bufs=3) as etp,
            tc.tile_pool(name="enat", bufs=2) as enp,
            tc.tile_pool(name="cqa", bufs=1) as cq,
            tc.tile_pool(name="small", bufs=4) as small,
        ):
            # ---------------- constants ----------------
            ident = constp.tile([128, 128], F32)
            masks.make_identity(nc, ident[:])
            eps_t = constp.tile([128, 1], F32)
            nc.gpsimd.memset(eps_t[:], EPS)

            # queue order matters: w1/w2/b1T go at the HEAD of the sync queue
            # (ahead of the xT stream) so the first hT matmuls and gelus are
            # unblocked within ~8us; everything else rides the scalar queue.
            b1T = constp.tile([128, KC], F32)
            nc.sync.dma_start(b1T[:], b1_d.rearrange("(c p) -> p c", p=128))
            w1_sb = constp.tile([128, KC, D], BF16)
            nc.sync.dma_start(w1_sb[:], w1_d.rearrange("p (c d) -> p c d", c=KC))
            w2_sb = constp.tile([128, KC, N], BF16)
            nc.sync.dma_start(w2_sb[:], w2_d.rearrange("p (c n) -> p c n", c=KC))

            def bcast_row(dst, src_1d):
                nc.scalar.dma_start(
                    dst, src_1d.rearrange("(o d) -> o d", o=1).broadcast_to([N, D])
                )

            g1_bc = constp.tile([N, D], F32)
            bcast_row(g1_bc[:], g1_d)
            be1_bc = constp.tile([N, D], F32)
            bcast_row(be1_bc[:], be1_d)
            g2_bc = constp.tile([N, D], F32)
            bcast_row(g2_bc[:], g2_d)
            be2_bc = constp.tile([N, D], F32)
            bcast_row(be2_bc[:], be2_d)
            bf_bc = constp.tile([N, D], F32)
            bcast_row(bf_bc[:], bfsum_d)
            wf_sb = constp.tile([128, 4, KC, D], BF16)
            nc.scalar.dma_start(
                wf_sb[:], wf_d.rearrange("p w (c d) -> p w c d", c=KC)
            )

            # ---------------- deferred CQA ----------------
            def emit_cqa(st, cqa_exp_sink, gelu_anchor):
                """Emit batch st['b']'s CQA. ACT exps are appended to
                cqa_exp_sink and (if gelu_anchor) ordered after those gelus."""
                feat_ps, epsc2, q_sb = st["feat_ps"], st["epsc2"], st["q_sb"]
                b = st["b"]

                feat_ln = cq.tile([N, D], F32, tag="feat_ln")
                _layer_norm(
                    nc, cq, small, feat_ps[:], g1_bc[:], be1_bc[:], feat_ln,
                    "ln1", epsc2[:], out_r=True,
                )

                qT_sb = st["qT_sb"]

                fT_ps = cqp.tile([128, KC * N], F32, tag="cq")
                for c in range(KC):
                    nc.tensor.matmul(
                        fT_ps[:, c * N : (c + 1) * N],
                        feat_ln[:, c * 128 : (c + 1) * 128],
                        ident[0:N, 0:N],
                        is_transpose=True, start=True, stop=True,
                    )
                fT_sb = cq.tile([128, KC * N], F32, tag="fT")
                nc.vector.tensor_copy(fT_sb[:], fT_ps[:])
                fTr_sb = cq.tile([128, KC * N], BF16, tag="fTr")
                nc.vector.tensor_copy(fTr_sb[:], fT_ps[:])

                # sim (both orientations, exact fp32)
                sim_ps = cqp.tile([N, L], F32, tag="cq")
                for c in range(KC):
                    nc.tensor.matmul(
                        sim_ps[:],
                        fT_sb[:, c * N : (c + 1) * N],
                        qT_sb[:, c * L : (c + 1) * L],
                        start=(c == 0), stop=(c == KC - 1),
                    )
                simT_ps = cqp.tile([L, N], F32, tag="cq")
                for c in range(KC):
                    nc.tensor.matmul(
                        simT_ps[:],
                        qT_sb[:, c * L : (c + 1) * L],
                        fT_sb[:, c * N : (c + 1) * N],
                        start=(c == 0), stop=(c == KC - 1),
                    )

                # row softmax over L
                ngr = small.tile([N, 1], F32, tag="ngr")
                nc.vector.reduce_max(ngr[:], sim_ps[:], axis=AX.X, negate=True)
                er = cq.tile([N, L], F32, tag="er")
                rs = small.tile([N, 1], F32, tag="rs")
                er_i = nc.scalar.activation(
                    er[:], sim_ps[:], AF.Exp, bias=ngr[:], accum_out=rs[:]
                )
                cqa_exp_sink.append(er_i.ins)
                rr = small.tile([N, 1], F32, tag="rr")
                nc.vector.reciprocal(rr[:], rs[:])
                sim_r = cq.tile([N, L], F32, tag="sim_r")
                nc.scalar.activation(sim_r[:], er[:], AF.Copy, scale=rr[:])

                # col softmax over N (free axis of simT)
                ngc = small.tile([L, 1], F32, tag="ngc")
                nc.vector.reduce_max(ngc[:], simT_ps[:], axis=AX.X, negate=True)
                ec = cq.tile([L, N], F32, tag="ec")
                cs = small.tile([L, 1], F32, tag="cs")
                ec_i = nc.scalar.activation(
                    ec[:], simT_ps[:], AF.Exp, bias=ngc[:], accum_out=cs[:]
                )
                cqa_exp_sink.append(ec_i.ins)
                rc = small.tile([L, 1], F32, tag="rc")
                nc.vector.reciprocal(rc[:], cs[:])
                sim_cT = cq.tile([L, N], F32, tag="sim_cT")
                nc.scalar.activation(sim_cT[:], ec[:], AF.Copy, scale=rc[:])

                if gelu_anchor:
                    for gi in gelu_anchor:
                        add_dep_helper(er_i.ins, gi, False, "act-table phase order")
                        add_dep_helper(ec_i.ins, gi, False, "act-table phase order")

                # transposes of the softmaxed maps
                srT_ps = cqp.tile([L, N], F32, tag="cq")
                nc.tensor.matmul(
                    srT_ps[:], sim_r[:], ident[0:N, 0:N], is_transpose=True,
                    start=True, stop=True,
                )
                sim_rT = cq.tile([L, N], F32, tag="sim_rT")
                nc.vector.tensor_copy(sim_rT[:].bitcast(F32R), srT_ps[:])
                sc_ps = cqp.tile([N, L], F32, tag="cq")
                nc.tensor.matmul(
                    sc_ps[:], sim_cT[:], ident[0:L, 0:L], is_transpose=True,
                    start=True, stop=True,
                )
                sim_c = cq.tile([N, L], F32, tag="sim_c")
                nc.vector.tensor_copy(sim_c[:].bitcast(F32R), sc_ps[:])

                # A = sim_r @ query ; M = sim_c.T @ feat ; Bm = sim_r @ M
                A_ps = cqp.tile([N, D], F32, tag="cq")
                nc.tensor.matmul(
                    A_ps[:], _r(sim_rT[:]), _r(q_sb[:]), start=True, stop=True
                )
                A_sb = cq.tile([N, D], F32, tag="A")
                nc.vector.tensor_copy(A_sb[:], A_ps[:])
                M_ps = cqp.tile([L, D], F32, tag="cq")
                nc.tensor.matmul(
                    M_ps[:], _r(sim_c[:]), _r(feat_ln[:]), start=True, stop=True
                )
                M_sb = cq.tile([L, D], F32, tag="M")
                nc.vector.tensor_copy(M_sb[:].bitcast(F32R), M_ps[:])
                B_ps = cqp.tile([N, D], F32, tag="cq")
                nc.tensor.matmul(
                    B_ps[:], _r(sim_rT[:]), _r(M_sb[:]), start=True, stop=True
                )

                fA = cq.tile([N, D], F32, tag="fA")
                nc.vector.tensor_mul(fA[:], feat_ln[:], A_sb[:])
                fB = cq.tile([N, D], F32, tag="fB")
                nc.vector.tensor_mul(fB[:], feat_ln[:], B_ps[:])

                def transpose_nd(src, tag):
                    t_ps = cqp.tile([128, KC * N], F32, tag="cq")
                    for c in range(KC):
                        nc.tensor.matmul(
                            t_ps[:, c * N : (c + 1) * N],
                            src[:, c * 128 : (c + 1) * 128],
                            ident[0:N, 0:N],
                            is_transpose=True, start=True, stop=True,
                        )
                    t_sb = cq.tile([128, KC * N], BF16, tag=tag)
                    nc.vector.tensor_copy(t_sb[:], t_ps[:])
                    return t_sb

                AT_sb = transpose_nd(A_sb, "AT")
                fAT_sb = transpose_nd(fA, "fAT")
                fBT_sb = transpose_nd(fB, "fBT")

                cat_ps = cqp.tile([N, D], F32, tag="cq")
                lhs_list = [fTr_sb, AT_sb, fAT_sb, fBT_sb]
                for c in range(KC):
                    for wi in range(4):
                        nc.tensor.matmul(
                            cat_ps[:],
                            lhs_list[wi][:, c * N : (c + 1) * N],
                            wf_sb[:, wi, c, :],
                            start=(c == 0 and wi == 0),
                            stop=(c == KC - 1 and wi == 3),
                        )
                cat_sb = cq.tile([N, D], F32, tag="cat")
                nc.vector.tensor_add(cat_sb[:], cat_ps[:], bf_bc[:])

                o_sb = cq.tile([N, D], F32, tag="o")
                _layer_norm(
                    nc, cq, small, cat_sb[:], g2_bc[:], be2_bc[:], o_sb, "ln2",
                    eps_t[0:N],
                )
                nc.scalar.dma_start(out_d[b], o_sb[:])

            # ---------------- windows ----------------
            # Window w interleaves, at macro granularity and with phase A
            # FIRST in emission order, phase A of batch w with phase B of
            # batch w-1. Batch w-1's exps are all data-ready at the window
            # start, so by the time the in-order PE stream reaches a phase-B
            # matmul its inputs exist - no stalls. CQA(w-1) is emitted after
            # the loop; its exps join the next window's exp phase.
            prev_gelus = []         # gelus of batch w-1
            prev_exps = []          # exps of batch w-2 (run in window w-1)
            prev_cqa_exps = []      # cqa exps of batch w-2
            st = None               # phase A state of batch w-1
            for w in range(BS + 1):
                gelus_w = []
                exps_w = []
                if w < BS:
                    selT_full = selnp.tile([N, T], BF16, tag="selT_full")
                    x_tiles = []
                    q_sb = qp.tile([L, D], F32, tag="q")
                    nc.scalar.dma_start(q_sb[:].bitcast(F32R), q_d[w])
                    qT_sb = qp.tile([128, KC * L], F32, tag="qT")
                    nc.scalar.dma_start(
                        qT_sb[:], qt_d[w].rearrange("p (c l) -> p c l", c=KC)
                    )
                if w > 0:
                    feat_ps = fpp.tile([N, D], F32, tag="feat")
                    dparts = small.tile([N, T // EXPCH], F32, tag="dparts")

                for m in range(NMACRO):
                    if w < BS:
                        # ---- phase A macro m of batch w ----
                        x_sb = xp.tile([128, JT, D], BF16, tag="x")
                        x_tiles.append(x_sb)
                        nc.gpsimd.dma_start(
                            x_sb[:],
                            x_d[w, m].rearrange("p (j d) -> p j d", j=JT),
                        )
                        xT_sb = xts.tile([128, KC, MACRO], BF16, tag="xT")
                        nc.sync.dma_start(
                            xT_sb[:],
                            xt_d[w, m].rearrange("p (c t) -> p c t", c=KC),
                        )
                        hT_sb = hts.tile([128, KC, MACRO], BF16, tag="hT")
                        for mm in range(KC):
                            h_ps = htp.tile([128, MACRO], F32, tag="h")
                            for k in range(KC):
                                nc.tensor.matmul(
                                    h_ps[:],
                                    w1_sb[:, k, mm * 128 : (mm + 1) * 128],
                                    xT_sb[:, k, :],
                                    start=(k == 0),
                                    stop=(k == KC - 1),
                                )
                            gi = nc.scalar.activation(
                                hT_sb[:, mm, :], h_ps[:], AF.Gelu,
                                bias=b1T[:, mm : mm + 1],
                            )
                            gelus_w.append(gi.ins)
                        se_ps = sep.tile([128, MACRO], F32, tag="sel")
                        selT = se_ps[0:N, :]
                        for k in range(KC):
                            nc.tensor.matmul(
                                selT,
                                w2_sb[:, k, :],
                                hT_sb[:, k, :],
                                start=(k == 0),
                                stop=(k == KC - 1),
                            )
                        nc.vector.tensor_copy(
                            selT_full[:, m * MACRO : (m + 1) * MACRO], selT
                        )
                    if w > 0:
                        # ---- phase B macro m of batch w-1 ----
                        if m % MPE == 0:
                            E_T = etp.tile([N, EXPCH], F32, tag="E_T")
                            exp_i = nc.scalar.activation(
                                E_T[:],
                                st["selT_full"][
                                    :, m * MACRO : m * MACRO + EXPCH
                                ],
                                AF.Exp,
                                accum_out=dparts[:, m // MPE : m // MPE + 1],
                            )
                            exps_w.append(exp_i.ins)
                            for gi in prev_gelus:
                                add_dep_helper(
                                    exp_i.ins, gi, False, "act-table order"
                                )
                        eoff = (m % MPE) * MACRO
                        en_ps = cqp.tile([128, JT * N], F32, tag="cq")
                        for j in range(JT):
                            nc.tensor.matmul(
                                en_ps[:, j * N : (j + 1) * N],
                                E_T[:, eoff + j * 128 : eoff + (j + 1) * 128],
                                ident[0:N, 0:N],
                                is_transpose=True,
                                start=True,
                                stop=True,
                            )
                        E_nat = enp.tile([128, JT * N], BF16, tag="E_nat")
                        nc.vector.tensor_copy(E_nat[:], en_ps[:])
                        for j in range(JT):
                            nc.tensor.matmul(
                                feat_ps[:],
                                E_nat[:, j * N : (j + 1) * N],
                                st["x_tiles"][m][:, j, :],
                                start=(m == 0 and j == 0),
                                stop=(m == NMACRO - 1 and j == JT - 1),
                            )

                # table order: gelus(w) after exps(w-1) (this window) and
                # after both exp groups of the previous window
                for gi in gelus_w:
                    for anchor in exps_w + prev_exps + prev_cqa_exps:
                        add_dep_helper(gi, anchor, False, "act-table order")

                cqa_exps = []
                if w > 0:
                    # eps * denom^2 keeps LN1 exact w/o softmax normalization
                    denom = small.tile([N, 1], F32, tag="denom")
                    nc.vector.reduce_sum(denom[:], dparts[:], axis=AX.X)
                    epsc2 = small.tile([N, 1], F32, tag="epsc2")
                    nc.vector.tensor_mul(epsc2[:], denom[:], denom[:])
                    nc.vector.tensor_scalar_mul(epsc2[:], epsc2[:], EPS)
                    st["feat_ps"] = feat_ps
                    st["epsc2"] = epsc2
                    emit_cqa(st, cqa_exps, gelus_w if w < BS else None)

                prev_gelus = gelus_w
                prev_exps = exps_w
                prev_cqa_exps = cqa_exps
                if w < BS:
                    st = {"b": w, "selT_full": selT_full, "x_tiles": x_tiles,
                          "q_sb": q_sb, "qT_sb": qT_sb}

    nc.compile()
    return nc


_NC_CACHE = None


def kernel(**inputs) -> np.ndarray:
    global _NC_CACHE
    if _NC_CACHE is None:
        _NC_CACHE = build_program()
    nc = _NC_CACHE

    def f32(a):
        return np.ascontiguousarray(np.asarray(a), dtype=np.float32)

    def bf(a):
        return np.asarray(a, dtype=np.float32).astype(ml_dtypes.bfloat16)

    x = bf(inputs["input"])                      # [B, T, D] bf16
    # token-major slabs: [B, NM, 128, JT*D], x_perm[b,m,p] = x[b, m*512+j*128+p, :]
    xp_ = np.ascontiguousarray(
        x.reshape(B, NMACRO, JT, 128, D).transpose(0, 1, 3, 2, 4)
    ).reshape(B, NMACRO, 128, JT * D)
    # feature-major slabs: [B, NM, 128, KC*MACRO],
    # xt_perm[b,m,p] = x[b, m*512:(m+1)*512, c*128+p].T
    xt_ = np.ascontiguousarray(
        x.reshape(B, NMACRO, MACRO, KC, 128).transpose(0, 1, 4, 3, 2)
    ).reshape(B, NMACRO, 128, KC * MACRO)
    q = f32(inputs["query"])
    # qt[b, p, c, l] = q[b, l, c*128+p]
    qt = np.ascontiguousarray(
        q.reshape(B, L, KC, 128).transpose(0, 3, 2, 1)
    ).reshape(B, 128, KC * L)
    w1p = np.ascontiguousarray(
        bf(inputs["w1"]).reshape(KC, 128, D).transpose(1, 0, 2)
    ).reshape(128, KC * D)
    w2p = np.ascontiguousarray(
        bf(inputs["w2"]).reshape(KC, 128, N).transpose(1, 0, 2)
    ).reshape(128, KC * N)
    wfp = np.ascontiguousarray(
        np.stack(
            [bf(inputs[f"wf{i}"]).reshape(KC, 128, D) for i in range(1, 5)]
        ).transpose(2, 0, 1, 3)
    ).reshape(128, 4, KC * D)
    bfsum = (
        f32(inputs["bf1"]) + f32(inputs["bf2"])
        + f32(inputs["bf3"]) + f32(inputs["bf4"])
    )
    shared = {
        "w1": w1p,
        "b1": f32(inputs["b1"]),
        "w2": w2p,
        "g1": f32(inputs["ln1_g"]),
        "be1": f32(inputs["ln1_b"]),
        "wf": wfp,
        "bfsum": bfsum,
        "g2": f32(inputs["ln2_g"]),
        "be2": f32(inputs["ln2_b"]),
    }
    in_maps = []
    for c in range(NCORES):
        m = dict(shared)
        m["x"] = xp_[c * BS : (c + 1) * BS]
        m["xt"] = xt_[c * BS : (c + 1) * BS]
        m["q"] = q[c * BS : (c + 1) * BS]
        m["qt"] = qt[c * BS : (c + 1) * BS]
        in_maps.append(m)

    res = bass_utils.run_bass_kernel_spmd(
        nc, in_maps, core_ids=list(range(NCORES)), trace=TRACE
    )
    if TRACE and res.exec_time_ns is not None:
        print(f"HW exec time: {res.exec_time_ns} ns")
    out = np.concatenate([res.results[c]["out"] for c in range(NCORES)], axis=0)
    return out


# revision 44
# speedup vs baseline: 1.1037x; 1.1037x over previous
"""CQATokenLearner fused Trainium2 kernel (v3).

Pure data parallel over batch: B=32 split as 4 batches per NeuronCore x 8 cores.
Weights replicated. Everything fused on-chip.

v3 design (v1 = 367us fp32r, v2 = 311us bf16):
  * all big-path data in bf16; x shipped twice (token-major for pooling,
    feature-major for the MLP) in HOST-PERMUTED layouts so every DMA
    descriptor is a contiguous 4KB block (v2's 1KB descriptors made the
    x stream the kernel tail).
  * sel logits FEATURE-major ([n, t]) - 512-row matmuls with a small
    stationary (v2's token-major variant quadrupled PE instruction count).
    Softmax denominators fall out of the exp's accum_out for free.
  * b2 dropped entirely (softmax over T is invariant to per-n shifts).
  * deferred CQA: batch b's CQA (LN1 -> sim -> softmaxes -> A/M/B -> cat ->
    LN2) is emitted after phase A of batch b+1 and its ACT exps are anchored
    into batch b+1's exp table window. This removes the per-batch PE bubble
    where next-batch gelus waited on the long serial CQA chain.
  * A/M/B matmuls in fp32r (1 cyc/row at 512 free) instead of fp32 (4x).
  * PSUM->SBUF staging on DVE; ACT only does gelu/exp/LN/softmax.

Exactness tricks kept from v1: no max-subtraction for the T-softmax (logits
are O(1)); softmax denominator cancels in LN1 except through eps, handled by
the eps*denom^2 bias; rsqrt via integer-magic seed + 3 Newton steps (DVE only,
no ACT table).
"""

import sys

sys.path.insert(0, "/opt/trn_rl_repo")

import numpy as np
import ml_dtypes

import concourse.bacc as bacc
import concourse.bass as bass
import concourse.mybir as mybir
import concourse.tile as tile
import concourse.masks as masks
import concourse.bass_utils as bass_utils
from concourse.tile_rust import add_dep_helper

F32 = mybir.dt.float32
F32R = mybir.dt.float32r
BF16 = mybir.dt.bfloat16
I32 = mybir.dt.int32
AF = mybir.ActivationFunctionType
ALU = mybir.AluOpType
AX = mybir.AxisListType

B, T, L, D, N = 32, 4096, 32, 512, 64
NCORES = 8
BS = B // NCORES          # batches per core
MACRO = 512               # tokens per macro tile
NMACRO = T // MACRO       # 8
JT = MACRO // 128         # 4 token sub-tiles per macro
KC = D // 128             # 4 chunks of the feature dim
EXPCH = 2048              # tokens per exp-instruction chunk
MPE = EXPCH // MACRO      # macros per exp chunk (4)
EPS = 1e-5

TRACE = False             # set by test harness for profiling


def _r(ap):
    """fp32r view of an fp32 AP (full-rate PE matmul mode)."""
    return ap.bitcast(F32R)


def _layer_norm(nc, cq, small, src, g_bc, b_bc, out_tile, pfx, eps_ap,
                out_r=False):
    # eps_ap: [P,1] per-partition eps bias added to var (LN1 passes
    # eps*denom^2 so the skipped softmax normalization stays exact)
    """LN over the free axis (D). src may be PSUM or SBUF, [P, D]."""
    P = src.shape[0]
    s = small.tile([P, 1], F32, tag="ln_s")
    nc.vector.reduce_sum(s[:], src, axis=AX.X)
    negmu = small.tile([P, 1], F32, tag="ln_negmu")
    nc.vector.tensor_scalar_mul(negmu[:], s[:], -1.0 / D)
    cent = cq.tile([P, D], F32, tag="ln_cent")
    nc.scalar.activation(cent[:], src, AF.Identity, bias=negmu[:])
    sq = cq.tile([P, D], F32, tag="ln_sq")
    vs = small.tile([P, 1], F32, tag="ln_vs")
    nc.scalar.activation(sq[:], cent[:], AF.Square, accum_out=vs[:])
    # veps = var + eps; rsqrt via DVE-only magic-number seed + 3 Newton steps
    veps = small.tile([P, 1], F32, tag="ln_veps")
    nc.scalar.activation(veps[:], vs[:], AF.Identity, scale=1.0 / D, bias=eps_ap)
    ish = small.tile([P, 1], I32, tag="ln_ish")
    nc.vector.tensor_scalar(
        ish[:], veps[:].bitcast(I32), 1, None, op0=ALU.arith_shift_right
    )
    fsh = small.tile([P, 1], F32, tag="ln_fsh")
    nc.vector.tensor_copy(fsh[:], ish[:])
    nc.vector.tensor_scalar(
        fsh[:], fsh[:], -1.0, float(0x5F3759DF), op0=ALU.mult, op1=ALU.add
    )
    rbits = small.tile([P, 1], I32, tag="ln_rbits")
    nc.vector.tensor_copy(rbits[:], fsh[:])
    r = rbits[:].bitcast(F32)
    for it in range(2):
        t1 = small.tile([P, 1], F32, tag=f"ln_nt{it}")
        nc.vector.tensor_mul(t1[:], veps[:], r)
        nc.vector.tensor_mul(t1[:], t1[:], r)
        nc.vector.tensor_scalar(t1[:], t1[:], -0.5, 1.5, op0=ALU.mult, op1=ALU.add)
        rn = small.tile([P, 1], F32, tag=f"ln_rn{it}")
        nc.vector.tensor_mul(rn[:], r, t1[:])
        r = rn[:]
    tmp = cq.tile([P, D], F32, tag="ln_tmp")
    nc.vector.scalar_tensor_tensor(
        tmp[:], cent[:], r, g_bc, op0=ALU.mult, op1=ALU.mult
    )
    dst = out_tile[:].bitcast(F32R) if out_r else out_tile[:]
    nc.vector.tensor_add(dst, tmp[:], b_bc)


def build_program():
    nc = bacc.Bacc(
        "TRN2",
        target_bir_lowering=False,
        debug=False,
        enable_asserts=False,
        num_devices=NCORES,
    )

    # host-permuted: x_d[b, m, p] = x[b, m*512 + j*128 + p, :] flat over (j, d)
    x_d = nc.dram_tensor(
        "x", [BS, NMACRO, 128, JT * D], BF16, kind="ExternalInput"
    ).ap()
    # host-permuted: xt_d[b, m, p] = xT[b, c*128 + p, m*512:(m+1)*512] flat (c, t)
    xt_d = nc.dram_tensor(
        "xt", [BS, NMACRO, 128, KC * MACRO], BF16, kind="ExternalInput"
    ).ap()
    q_d = nc.dram_tensor("q", [BS, L, D], F32R, kind="ExternalInput").ap()
    # host-transposed q: qt_d[b, p] = q[b, :, c*128+p].T flat over (c, l)
    qt_d = nc.dram_tensor("qt", [BS, 128, KC * L], F32, kind="ExternalInput").ap()
    # host-permuted: w1_d[p] = w1[c*128 + p, :] flat over (c, d)
    w1_d = nc.dram_tensor("w1", [128, KC * D], BF16, kind="ExternalInput").ap()
    b1_d = nc.dram_tensor("b1", [D], F32, kind="ExternalInput").ap()
    w2_d = nc.dram_tensor("w2", [128, KC * N], BF16, kind="ExternalInput").ap()
    g1_d = nc.dram_tensor("g1", [D], F32, kind="ExternalInput").ap()
    be1_d = nc.dram_tensor("be1", [D], F32, kind="ExternalInput").ap()
    wf_d = nc.dram_tensor(
        "wf", [128, 4, KC * D], BF16, kind="ExternalInput"
    ).ap()
    bfsum_d = nc.dram_tensor("bfsum", [D], F32, kind="ExternalInput").ap()
    g2_d = nc.dram_tensor("g2", [D], F32, kind="ExternalInput").ap()
    be2_d = nc.dram_tensor("be2", [D], F32, kind="ExternalInput").ap()
    out_d = nc.dram_tensor("out", [BS, N, D], F32, kind="ExternalOutput").ap()

    with tile.TileContext(nc) as tc:
        with (
            tc.tile_pool(name="const", bufs=1) as constp,
            tc.tile_pool(name="xin", bufs=12) as xp,
            tc.tile_pool(name="xt_sb", bufs=6) as xts,
            tc.tile_pool(name="ht_sb", bufs=2) as hts,
            tc.tile_pool(name="qin", bufs=2) as qp,
            tc.tile_pool(name="ht_ps", bufs=4, space="PSUM") as htp,
            tc.tile_pool(name="sel_ps", bufs=1, space="PSUM") as sep,
            tc.tile_pool(name="feat_ps", bufs=1, space="PSUM") as fpp,
            tc.tile_pool(name="cq_ps", bufs=2, space="PSUM") as cqp,
            tc.tile_pool(name="seln", bufs=2) as selnp,
            tc.tile_pool(name="et", bufs=3) as etp,
            tc.tile_pool(name="enat", bufs=2) as enp,
            tc.tile_pool(name="cqa", bufs=1) as cq,
            tc.tile_pool(name="small", bufs=4) as small,
        ):
            # ---------------- constants ----------------
            ident = constp.tile([128, 128], F32)
            masks.make_identity(nc, ident[:])
            eps_t = constp.tile([128, 1], F32)
            nc.gpsimd.memset(eps_t[:], EPS)

            # queue order matters: w1/w2/b1T go at the HEAD of the sync queue
            # (ahead of the xT stream) so the first hT matmuls and gelus are
            # unblocked within ~8us; everything else rides the scalar queue.
            b1T = constp.tile([128, KC], F32)
            nc.sync.dma_start(b1T[:], b1_d.rearrange("(c p) -> p c", p=128))
            w1_sb = constp.tile([128, KC, D], BF16)
            nc.sync.dma_start(w1_sb[:], w1_d.rearrange("p (c d) -> p c d", c=KC))
            w2_sb = constp.tile([128, KC, N], BF16)
            nc.sync.dma_start(w2_sb[:], w2_d.rearrange("p (c n) -> p c n", c=KC))

            def bcast_row(dst, src_1d):
                nc.scalar.dma_start(
                    dst, src_1d.rearrange("(o d) -> o d", o=1).broadcast_to([N, D])
                )

            g1_bc = constp.tile([N, D], F32)
            bcast_row(g1_bc[:], g1_d)
            be1_bc = constp.tile([N, D], F32)
            bcast_row(be1_bc[:], be1_d)
            g2_bc = constp.tile([N, D], F32)
            bcast_row(g2_bc[:], g2_d)
            be2_bc = constp.tile([N, D], F32)
            bcast_row(be2_bc[:], be2_d)
            bf_bc = constp.tile([N, D], F32)
            bcast_row(bf_bc[:], bfsum_d)
            wf_sb = constp.tile([128, 4, KC, D], BF16)
            nc.scalar.dma_start(
                wf_sb[:], wf_d.rearrange("p w (c d) -> p w c d", c=KC)
            )

            # ---------------- deferred CQA ----------------
            def emit_cqa(st, cqa_exp_sink, gelu_anchor):
                """Emit batch st['b']'s CQA. ACT exps are appended to
                cqa_exp_sink and (if gelu_anchor) ordered after those gelus."""
                feat_ps, epsc2, q_sb = st["feat_ps"], st["epsc2"], st["q_sb"]
                b = st["b"]

                feat_ln = cq.tile([N, D], F32, tag="feat_ln")
                _layer_norm(
                    nc, cq, small, feat_ps[:], g1_bc[:], be1_bc[:], feat_ln,
                    "ln1", epsc2[:], out_r=True,
                )

                qT_sb = st["qT_sb"]

                fT_ps = cqp.tile([128, KC * N], F32, tag="cq")
                for c in range(KC):
                    nc.tensor.matmul(
                        fT_ps[:, c * N : (c + 1) * N],
                        feat_ln[:, c * 128 : (c + 1) * 128],
                        ident[0:N, 0:N],
                        is_transpose=True, start=True, stop=True,
                    )
                fT_sb = cq.tile([128, KC * N], F32, tag="fT")
                nc.vector.tensor_copy(fT_sb[:], fT_ps[:])
                fTr_sb = cq.tile([128, KC * N], BF16, tag="fTr")
                nc.vector.tensor_copy(fTr_sb[:], fT_ps[:])

                # sim (both orientations, exact fp32)
                sim_ps = cqp.tile([N, L], F32, tag="cq")
                for c in range(KC):
                    nc.tensor.matmul(
                        sim_ps[:],
                        fT_sb[:, c * N : (c + 1) * N],
                        qT_sb[:, c * L : (c + 1) * L],
                        start=(c == 0), stop=(c == KC - 1),
                    )
                simT_ps = cqp.tile([L, N], F32, tag="cq")
                for c in range(KC):
                    nc.tensor.matmul(
                        simT_ps[:],
                        qT_sb[:, c * L : (c + 1) * L],
                        fT_sb[:, c * N : (c + 1) * N],
                        start=(c == 0), stop=(c == KC - 1),
                    )

                # row softmax over L
                ngr = small.tile([N, 1], F32, tag="ngr")
                nc.vector.reduce_max(ngr[:], sim_ps[:], axis=AX.X, negate=True)
                er = cq.tile([N, L], F32, tag="er")
                rs = small.tile([N, 1], F32, tag="rs")
                er_i = nc.scalar.activation(
                    er[:], sim_ps[:], AF.Exp, bias=ngr[:], accum_out=rs[:]
                )
                cqa_exp_sink.append(er_i.ins)
                rr = small.tile([N, 1], F32, tag="rr")
                nc.vector.reciprocal(rr[:], rs[:])
                sim_r = cq.tile([N, L], F32, tag="sim_r")
                nc.scalar.activation(sim_r[:], er[:], AF.Copy, scale=rr[:])

                # col softmax over N (free axis of simT)
                ngc = small.tile([L, 1], F32, tag="ngc")
                nc.vector.reduce_max(ngc[:], simT_ps[:], axis=AX.X, negate=True)
                ec = cq.tile([L, N], F32, tag="ec")
                cs = small.tile([L, 1], F32, tag="cs")
                ec_i = nc.scalar.activation(
                    ec[:], simT_ps[:], AF.Exp, bias=ngc[:], accum_out=cs[:]
                )
                cqa_exp_sink.append(ec_i.ins)
                rc = small.tile([L, 1], F32, tag="rc")
                nc.vector.reciprocal(rc[:], cs[:])
                sim_cT = cq.tile([L, N], F32, tag="sim_cT")
                nc.scalar.activation(sim_cT[:], ec[:], AF.Copy, scale=rc[:])

                if gelu_anchor:
                    for gi in gelu_anchor:
                        add_dep_helper(er_i.ins, gi, False, "act-table phase order")
                        add_dep_helper(ec_i.ins, gi, False, "act-table phase order")

                # transposes of the softmaxed maps
                srT_ps = cqp.tile([L, N], F32, tag="cq")
                nc.tensor.matmul(
                    srT_ps[:], sim_r[:], ident[0:N, 0:N], is_transpose=True,
                    start=True, stop=True,
                )
                sim_rT = cq.tile([L, N], F32, tag="sim_rT")
                nc.vector.tensor_copy(sim_rT[:].bitcast(F32R), srT_ps[:])
                sc_ps = cqp.tile([N, L], F32, tag="cq")
                nc.tensor.matmul(
                    sc_ps[:], sim_cT[:], ident[0:L, 0:L], is_transpose=True,
                    start=True, stop=True,
                )
                sim_c = cq.tile([N, L], F32, tag="sim_c")
                nc.vector.tensor_copy(sim_c[:].bitcast(F32R), sc_ps[:])

                # A = sim_r @ query ; M = sim_c.T @ feat ; Bm = sim_r @ M
                A_ps = cqp.tile([N, D], F32, tag="cq")
                nc.tensor.matmul(
                    A_ps[:], _r(sim_rT[:]), _r(q_sb[:]), start=True, stop=True
                )
                A_sb = cq.tile([N, D], F32, tag="A")
                nc.vector.tensor_copy(A_sb[:], A_ps[:])
                M_ps = cqp.tile([L, D], F32, tag="cq")
                nc.tensor.matmul(
                    M_ps[:], _r(sim_c[:]), _r(feat_ln[:]), start=True, stop=True
                )
                M_sb = cq.tile([L, D], F32, tag="M")
                nc.vector.tensor_copy(M_sb[:].bitcast(F32R), M_ps[:])
                B_ps = cqp.tile([N, D], F32, tag="cq")
                nc.tensor.matmul(
                    B_ps[:], _r(sim_rT[:]), _r(M_sb[:]), start=True, stop=True
                )

                fA = cq.tile([N, D], F32, tag="fA")
                nc.vector.tensor_mul(fA[:], feat_ln[:], A_sb[:])
                fB = cq.tile([N, D], F32, tag="fB")
                nc.vector.tensor_mul(fB[:], feat_ln[:], B_ps[:])

                def transpose_nd(src, tag):
                    t_ps = cqp.tile([128, KC * N], F32, tag="cq")
                    for c in range(KC):
                        nc.tensor.matmul(
                            t_ps[:, c * N : (c + 1) * N],
                            src[:, c * 128 : (c + 1) * 128],
                            ident[0:N, 0:N],
                            is_transpose=True, start=True, stop=True,
                        )
                    t_sb = cq.tile([128, KC * N], BF16, tag=tag)
                    nc.vector.tensor_copy(t_sb[:], t_ps[:])
                    return t_sb

                AT_sb = transpose_nd(A_sb, "AT")
                fAT_sb = transpose_nd(fA, "fAT")
                fBT_sb = transpose_nd(fB, "fBT")

                cat_ps = cqp.tile([N, D], F32, tag="cq")
                lhs_list = [fTr_sb, AT_sb, fAT_sb, fBT_sb]
                for c in range(KC):
                    for wi in range(4):
                        nc.tensor.matmul(
                            cat_ps[:],
                            lhs_list[wi][:, c * N : (c + 1) * N],
                            wf_sb[:, wi, c, :],
                            start=(c == 0 and wi == 0),
                            stop=(c == KC - 1 and wi == 3),
                        )
                cat_sb = cq.tile([N, D], F32, tag="cat")
                nc.vector.tensor_add(cat_sb[:], cat_ps[:], bf_bc[:])

                o_sb = cq.tile([N, D], F32, tag="o")
                _layer_norm(
                    nc, cq, small, cat_sb[:], g2_bc[:], be2_bc[:], o_sb, "ln2",
                    eps_t[0:N],
                )
                nc.scalar.dma_start(out_d[b], o_sb[:])

            # ---------------- per batch ----------------
            # Window b: [phase A(b)] -> [CQA(b-1)] -> [phase B(b)].
            # ACT table phases: [gelus(b)] [exps(b) + cqa-exps(b-1)] repeat,
            # enforced with explicit dep edges = 2 table loads per batch.
            prev_exp_anchors = []   # exps(b-1) + cqa exps of b-2
            pending = None          # state of batch b-1 awaiting CQA
            for b in range(BS):
                gelus_b = []
                # ===== phase A (gelu table set): xT -> hT -> selT =====
                selT_full = selnp.tile([N, T], BF16, tag="selT_full")
                x_tiles = []
                q_sb = qp.tile([L, D], F32, tag="q")
                nc.scalar.dma_start(q_sb[:].bitcast(F32R), q_d[b])
                qT_sb = qp.tile([128, KC * L], F32, tag="qT")
                nc.scalar.dma_start(
                    qT_sb[:], qt_d[b].rearrange("p (c l) -> p c l", c=KC)
                )
                for m in range(NMACRO):
                    x_sb = xp.tile([128, JT, D], BF16, tag="x")
                    x_tiles.append(x_sb)
                    nc.gpsimd.dma_start(
                        x_sb[:], x_d[b, m].rearrange("p (j d) -> p j d", j=JT)
                    )
                    xT_sb = xts.tile([128, KC, MACRO], BF16, tag="xT")
                    nc.sync.dma_start(
                        xT_sb[:], xt_d[b, m].rearrange("p (c t) -> p c t", c=KC)
                    )
                    hT_sb = hts.tile([128, KC, MACRO], BF16, tag="hT")
                    for mm in range(KC):
                        h_ps = htp.tile([128, MACRO], F32, tag="h")
                        for k in range(KC):
                            nc.tensor.matmul(
                                h_ps[:],
                                w1_sb[:, k, mm * 128 : (mm + 1) * 128],
                                xT_sb[:, k, :],
                                start=(k == 0),
                                stop=(k == KC - 1),
                            )
                        gi = nc.scalar.activation(
                            hT_sb[:, mm, :], h_ps[:], AF.Gelu,
                            bias=b1T[:, mm : mm + 1],
                        )
                        gelus_b.append(gi.ins)
                    se_ps = sep.tile([128, MACRO], F32, tag="sel")
                    selT = se_ps[0:N, :]
                    for k in range(KC):
                        nc.tensor.matmul(
                            selT,
                            w2_sb[:, k, :],
                            hT_sb[:, k, :],
                            start=(k == 0),
                            stop=(k == KC - 1),
                        )
                    nc.vector.tensor_copy(
                        selT_full[:, m * MACRO : (m + 1) * MACRO], selT
                    )

                # table phase order: gelus after previous window's exps
                for gi in gelus_b:
                    for anchor in prev_exp_anchors:
                        add_dep_helper(gi, anchor, False, "act-table order")

                # ----- deferred CQA of batch b-1 (exps join this window) -----
                cqa_exps = []
                if pending is not None:
                    emit_cqa(pending, cqa_exps, gelus_b)

                # ===== phase B (exp table set) =====
                feat_ps = fpp.tile([N, D], F32, tag="feat")
                dparts = small.tile([N, T // EXPCH], F32, tag="dparts")
                exps_b = []
                for m in range(NMACRO):
                    if m % MPE == 0:
                        E_T = etp.tile([N, EXPCH], F32, tag="E_T")
                        exp_i = nc.scalar.activation(
                            E_T[:],
                            selT_full[:, m * MACRO : m * MACRO + EXPCH],
                            AF.Exp,
                            accum_out=dparts[:, m // MPE : m // MPE + 1],
                        )
                        exps_b.append(exp_i.ins)
                        for gi in gelus_b:
                            add_dep_helper(exp_i.ins, gi, False, "act-table order")
                    eoff = (m % MPE) * MACRO
                    en_ps = cqp.tile([128, JT * N], F32, tag="cq")
                    for j in range(JT):
                        nc.tensor.matmul(
                            en_ps[:, j * N : (j + 1) * N],
                            E_T[:, eoff + j * 128 : eoff + (j + 1) * 128],
                            ident[0:N, 0:N],
                            is_transpose=True,
                            start=True,
                            stop=True,
                        )
                    E_nat = enp.tile([128, JT * N], BF16, tag="E_nat")
                    nc.vector.tensor_copy(E_nat[:], en_ps[:])
                    for j in range(JT):
                        nc.tensor.matmul(
                            feat_ps[:],
                            E_nat[:, j * N : (j + 1) * N],
                            x_tiles[m][:, j, :],
                            start=(m == 0 and j == 0),
                            stop=(m == NMACRO - 1 and j == JT - 1),
                        )

                # eps * denom^2 keeps LN1 exact w/o softmax normalization
                denom = small.tile([N, 1], F32, tag="denom")
                nc.vector.reduce_sum(denom[:], dparts[:], axis=AX.X)
                epsc2 = small.tile([N, 1], F32, tag="epsc2")
                nc.vector.tensor_mul(epsc2[:], denom[:], denom[:])
                nc.vector.tensor_scalar_mul(epsc2[:], epsc2[:], EPS)

                prev_exp_anchors = exps_b + cqa_exps
                pending = {"b": b, "feat_ps": feat_ps, "epsc2": epsc2,
                           "q_sb": q_sb, "qT_sb": qT_sb}

            # final batch's CQA (stays in the last exp window)
            tail_exps = []
            emit_cqa(pending, tail_exps, None)

    nc.compile()
    return nc


_NC_CACHE = None


def kernel(**inputs) -> np.ndarray:
    global _NC_CACHE
    if _NC_CACHE is None:
        _NC_CACHE = build_program()
    nc = _NC_CACHE

    def f32(a):
        return np.ascontiguousarray(np.asarray(a), dtype=np.float32)

    def bf(a):
        return np.asarray(a, dtype=np.float32).astype(ml_dtypes.bfloat16)

    x = bf(inputs["input"])                      # [B, T, D] bf16
    # token-major slabs: [B, NM, 128, JT*D], x_perm[b,m,p] = x[b, m*512+j*128+p, :]
    xp_ = np.ascontiguousarray(
        x.reshape(B, NMACRO, JT, 128, D).transpose(0, 1, 3, 2, 4)
    ).reshape(B, NMACRO, 128, JT * D)
    # feature-major slabs: [B, NM, 128, KC*MACRO],
    # xt_perm[b,m,p] = x[b, m*512:(m+1)*512, c*128+p].T
    xt_ = np.ascontiguousarray(
        x.reshape(B, NMACRO, MACRO, KC, 128).transpose(0, 1, 4, 3, 2)
    ).reshape(B, NMACRO, 128, KC * MACRO)
    q = f32(inputs["query"])
    # qt[b, p, c, l] = q[b, l, c*128+p]
    qt = np.ascontiguousarray(
        q.reshape(B, L, KC, 128).transpose(0, 3, 2, 1)
    ).reshape(B, 128, KC * L)
    w1p = np.ascontiguousarray(
        bf(inputs["w1"]).reshape(KC, 128, D).transpose(1, 0, 2)
    ).reshape(128, KC * D)
    w2p = np.ascontiguousarray(
        bf(inputs["w2"]).reshape(KC, 128, N).transpose(1, 0, 2)
    ).reshape(128, KC * N)
    wfp = np.ascontiguousarray(
        np.stack(
            [bf(inputs[f"wf{i}"]).reshape(KC, 128, D) for i in range(1, 5)]
        ).transpose(2, 0, 1, 3)
    ).reshape(128, 4, KC * D)
    bfsum = (
        f32(inputs["bf1"]) + f32(inputs["bf2"])
        + f32(inputs["bf3"]) + f32(inputs["bf4"])
    )
    shared = {
        "w1": w1p,
        "b1": f32(inputs["b1"]),
        "w2": w2p,
        "g1": f32(inputs["ln1_g"]),
        "be1": f32(inputs["ln1_b"]),
        "wf": wfp,
        "bfsum": bfsum,
        "g2": f32(inputs["ln2_g"]),
        "be2": f32(inputs["ln2_b"]),
    }
    in_maps = []
    for c in range(NCORES):
        m = dict(shared)
        m["x"] = xp_[c * BS : (c + 1) * BS]
        m["xt"] = xt_[c * BS : (c + 1) * BS]
        m["q"] = q[c * BS : (c + 1) * BS]
        m["qt"] = qt[c * BS : (c + 1) * BS]
        in_maps.append(m)

    res = bass_utils.run_bass_kernel_spmd(
        nc, in_maps, core_ids=list(range(NCORES)), trace=TRACE
    )
    if TRACE and res.exec_time_ns is not None:
        print(f"HW exec time: {res.exec_time_ns} ns")
    out = np.concatenate([res.results[c]["out"] for c in range(NCORES)], axis=0)
    return out


# revision 45
# speedup vs baseline: 1.1464x; 1.0387x over previous
"""CQATokenLearner fused Trainium2 kernel (v3).

Pure data parallel over batch: B=32 split as 4 batches per NeuronCore x 8 cores.
Weights replicated. Everything fused on-chip.

v3 design (v1 = 367us fp32r, v2 = 311us bf16):
  * all big-path data in bf16; x shipped twice (token-major for pooling,
    feature-major for the MLP) in HOST-PERMUTED layouts so every DMA
    descriptor is a contiguous 4KB block (v2's 1KB descriptors made the
    x stream the kernel tail).
  * sel logits FEATURE-major ([n, t]) - 512-row matmuls with a small
    stationary (v2's token-major variant quadrupled PE instruction count).
    Softmax denominators fall out of the exp's accum_out for free.
  * b2 dropped entirely (softmax over T is invariant to per-n shifts).
  * deferred CQA: batch b's CQA (LN1 -> sim -> softmaxes -> A/M/B -> cat ->
    LN2) is emitted after phase A of batch b+1 and its ACT exps are anchored
    into batch b+1's exp table window. This removes the per-batch PE bubble
    where next-batch gelus waited on the long serial CQA chain.
  * A/M/B matmuls in fp32r (1 cyc/row at 512 free) instead of fp32 (4x).
  * PSUM->SBUF staging on DVE; ACT only does gelu/exp/LN/softmax.

Exactness tricks kept from v1: no max-subtraction for the T-softmax (logits
are O(1)); softmax denominator cancels in LN1 except through eps, handled by
the eps*denom^2 bias; rsqrt via integer-magic seed + 3 Newton steps (DVE only,
no ACT table).
"""

import sys

sys.path.insert(0, "/opt/trn_rl_repo")

import numpy as np
import ml_dtypes

import concourse.bacc as bacc
import concourse.bass as bass
import concourse.mybir as mybir
import concourse.tile as tile
import concourse.masks as masks
import concourse.bass_utils as bass_utils
from concourse.tile_rust import add_dep_helper

F32 = mybir.dt.float32
F32R = mybir.dt.float32r
BF16 = mybir.dt.bfloat16
I32 = mybir.dt.int32
AF = mybir.ActivationFunctionType
ALU = mybir.AluOpType
AX = mybir.AxisListType

B, T, L, D, N = 32, 4096, 32, 512, 64
NCORES = 8
BS = B // NCORES          # batches per core
MACRO = 512               # tokens per macro tile
NMACRO = T // MACRO       # 8
JT = MACRO // 128         # 4 token sub-tiles per macro
KC = D // 128             # 4 chunks of the feature dim
EXPCH = 2048              # tokens per exp-instruction chunk
MPE = EXPCH // MACRO      # macros per exp chunk (4)
EPS = 1e-5

TRACE = False             # set by test harness for profiling


def _r(ap):
    """fp32r view of an fp32 AP (full-rate PE matmul mode)."""
    return ap.bitcast(F32R)


def _layer_norm(nc, cq, small, src, g_bc, b_bc, out_tile, pfx, eps_ap,
                out_r=False):
    # eps_ap: [P,1] per-partition eps bias added to var (LN1 passes
    # eps*denom^2 so the skipped softmax normalization stays exact)
    """LN over the free axis (D). src may be PSUM or SBUF, [P, D]."""
    P = src.shape[0]
    s = small.tile([P, 1], F32, tag="ln_s")
    nc.vector.reduce_sum(s[:], src, axis=AX.X)
    negmu = small.tile([P, 1], F32, tag="ln_negmu")
    nc.vector.tensor_scalar_mul(negmu[:], s[:], -1.0 / D)
    cent = cq.tile([P, D], F32, tag="ln_cent")
    nc.scalar.activation(cent[:], src, AF.Identity, bias=negmu[:])
    sq = cq.tile([P, D], F32, tag="ln_sq")
    vs = small.tile([P, 1], F32, tag="ln_vs")
    nc.scalar.activation(sq[:], cent[:], AF.Square, accum_out=vs[:])
    # veps = var + eps; rsqrt via DVE-only magic-number seed + 3 Newton steps
    veps = small.tile([P, 1], F32, tag="ln_veps")
    nc.scalar.activation(veps[:], vs[:], AF.Identity, scale=1.0 / D, bias=eps_ap)
    ish = small.tile([P, 1], I32, tag="ln_ish")
    nc.vector.tensor_scalar(
        ish[:], veps[:].bitcast(I32), 1, None, op0=ALU.arith_shift_right
    )
    fsh = small.tile([P, 1], F32, tag="ln_fsh")
    nc.vector.tensor_copy(fsh[:], ish[:])
    nc.vector.tensor_scalar(
        fsh[:], fsh[:], -1.0, float(0x5F3759DF), op0=ALU.mult, op1=ALU.add
    )
    rbits = small.tile([P, 1], I32, tag="ln_rbits")
    nc.vector.tensor_copy(rbits[:], fsh[:])
    r = rbits[:].bitcast(F32)
    for it in range(2):
        t1 = small.tile([P, 1], F32, tag=f"ln_nt{it}")
        nc.vector.tensor_mul(t1[:], veps[:], r)
        nc.vector.tensor_mul(t1[:], t1[:], r)
        nc.vector.tensor_scalar(t1[:], t1[:], -0.5, 1.5, op0=ALU.mult, op1=ALU.add)
        rn = small.tile([P, 1], F32, tag=f"ln_rn{it}")
        nc.vector.tensor_mul(rn[:], r, t1[:])
        r = rn[:]
    tmp = cq.tile([P, D], F32, tag="ln_tmp")
    nc.vector.scalar_tensor_tensor(
        tmp[:], cent[:], r, g_bc, op0=ALU.mult, op1=ALU.mult
    )
    dst = out_tile[:].bitcast(F32R) if out_r else out_tile[:]
    nc.vector.tensor_add(dst, tmp[:], b_bc)


def build_program():
    nc = bacc.Bacc(
        "TRN2",
        target_bir_lowering=False,
        debug=False,
        enable_asserts=False,
        num_devices=NCORES,
    )

    # host-permuted: x_d[b, m, p] = x[b, m*512 + j*128 + p, :] flat over (j, d)
    x_d = nc.dram_tensor(
        "x", [BS, NMACRO, 128, JT * D], BF16, kind="ExternalInput"
    ).ap()
    # host-permuted: xt_d[b, m, p] = xT[b, c*128 + p, m*512:(m+1)*512] flat (c, t)
    xt_d = nc.dram_tensor(
        "xt", [BS, NMACRO, 128, KC * MACRO], BF16, kind="ExternalInput"
    ).ap()
    q_d = nc.dram_tensor("q", [BS, L, D], F32R, kind="ExternalInput").ap()
    # host-transposed q: qt_d[b, p] = q[b, :, c*128+p].T flat over (c, l)
    qt_d = nc.dram_tensor("qt", [BS, 128, KC * L], F32, kind="ExternalInput").ap()
    # host-permuted: w1_d[p] = w1[c*128 + p, :] flat over (c, d)
    w1_d = nc.dram_tensor("w1", [128, KC * D], BF16, kind="ExternalInput").ap()
    b1_d = nc.dram_tensor("b1", [D], F32, kind="ExternalInput").ap()
    w2_d = nc.dram_tensor("w2", [128, KC * N], BF16, kind="ExternalInput").ap()
    g1_d = nc.dram_tensor("g1", [D], F32, kind="ExternalInput").ap()
    be1_d = nc.dram_tensor("be1", [D], F32, kind="ExternalInput").ap()
    wf_d = nc.dram_tensor(
        "wf", [128, 4, KC * D], BF16, kind="ExternalInput"
    ).ap()
    bfsum_d = nc.dram_tensor("bfsum", [D], F32, kind="ExternalInput").ap()
    g2_d = nc.dram_tensor("g2", [D], F32, kind="ExternalInput").ap()
    be2_d = nc.dram_tensor("be2", [D], F32, kind="ExternalInput").ap()
    out_d = nc.dram_tensor("out", [BS, N, D], F32, kind="ExternalOutput").ap()

    with tile.TileContext(nc) as tc:
        with (
            tc.tile_pool(name="const", bufs=1) as constp,
            tc.tile_pool(name="xin", bufs=12) as xp,
            tc.tile_pool(name="xt_sb", bufs=6) as xts,
            tc.tile_pool(name="ht_sb", bufs=2) as hts,
            tc.tile_pool(name="qin", bufs=2) as qp,
            tc.tile_pool(name="ht_ps", bufs=4, space="PSUM") as htp,
            tc.tile_pool(name="sel_ps", bufs=1, space="PSUM") as sep,
            tc.tile_pool(name="feat_ps", bufs=1, space="PSUM") as fpp,
            tc.tile_pool(name="cq_ps", bufs=2, space="PSUM") as cqp,
            tc.tile_pool(name="seln", bufs=2) as selnp,
            tc.tile_pool(name="et", bufs=3) as etp,
            tc.tile_pool(name="enat", bufs=2) as enp,
            tc.tile_pool(name="cqa", bufs=1) as cq,
            tc.tile_pool(name="small", bufs=4) as small,
        ):
            # ---------------- constants ----------------
            ident = constp.tile([128, 128], F32)
            masks.make_identity(nc, ident[:])
            eps_t = constp.tile([128, 1], F32)
            nc.gpsimd.memset(eps_t[:], EPS)

            # queue order matters: w1/w2/b1T go at the HEAD of the sync queue
            # (ahead of the xT stream) so the first hT matmuls and gelus are
            # unblocked within ~8us; everything else rides the scalar queue.
            b1T = constp.tile([128, KC], F32)
            nc.sync.dma_start(b1T[:], b1_d.rearrange("(c p) -> p c", p=128))
            w1_sb = constp.tile([128, KC, D], BF16)
            nc.sync.dma_start(w1_sb[:], w1_d.rearrange("p (c d) -> p c d", c=KC))
            w2_sb = constp.tile([128, KC, N], BF16)
            nc.sync.dma_start(w2_sb[:], w2_d.rearrange("p (c n) -> p c n", c=KC))

            def bcast_row(dst, src_1d):
                nc.scalar.dma_start(
                    dst, src_1d.rearrange("(o d) -> o d", o=1).broadcast_to([N, D])
                )

            g1_bc = constp.tile([N, D], F32)
            bcast_row(g1_bc[:], g1_d)
            be1_bc = constp.tile([N, D], F32)
            bcast_row(be1_bc[:], be1_d)
            g2_bc = constp.tile([N, D], F32)
            bcast_row(g2_bc[:], g2_d)
            be2_bc = constp.tile([N, D], F32)
            bcast_row(be2_bc[:], be2_d)
            bf_bc = constp.tile([N, D], F32)
            bcast_row(bf_bc[:], bfsum_d)
            wf_sb = constp.tile([128, 4, KC, D], BF16)
            nc.scalar.dma_start(
                wf_sb[:], wf_d.rearrange("p w (c d) -> p w c d", c=KC)
            )

            # ---------------- deferred CQA ----------------
            def emit_cqa(st, cqa_exp_sink, gelu_anchor):
                """Emit batch st['b']'s CQA. ACT exps are appended to
                cqa_exp_sink and (if gelu_anchor) ordered after those gelus."""
                feat_ps, epsc2, q_sb = st["feat_ps"], st["epsc2"], st["q_sb"]
                b = st["b"]

                feat_ln = cq.tile([N, D], F32, tag="feat_ln")
                _layer_norm(
                    nc, cq, small, feat_ps[:], g1_bc[:], be1_bc[:], feat_ln,
                    "ln1", epsc2[:], out_r=True,
                )

                qT_sb = st["qT_sb"]

                fT_ps = cqp.tile([128, KC * N], F32, tag="cq")
                for c in range(KC):
                    nc.tensor.matmul(
                        fT_ps[:, c * N : (c + 1) * N],
                        feat_ln[:, c * 128 : (c + 1) * 128],
                        ident[0:N, 0:N],
                        is_transpose=True, start=True, stop=True,
                    )
                fT_sb = cq.tile([128, KC * N], F32, tag="fT")
                nc.vector.tensor_copy(fT_sb[:], fT_ps[:])
                fTr_sb = cq.tile([128, KC * N], BF16, tag="fTr")
                nc.vector.tensor_copy(fTr_sb[:], fT_ps[:])

                # sim (both orientations, exact fp32)
                sim_ps = cqp.tile([N, L], F32, tag="cq")
                for c in range(KC):
                    nc.tensor.matmul(
                        sim_ps[:],
                        fT_sb[:, c * N : (c + 1) * N],
                        qT_sb[:, c * L : (c + 1) * L],
                        start=(c == 0), stop=(c == KC - 1),
                    )
                simT_ps = cqp.tile([L, N], F32, tag="cq")
                for c in range(KC):
                    nc.tensor.matmul(
                        simT_ps[:],
                        qT_sb[:, c * L : (c + 1) * L],
                        fT_sb[:, c * N : (c + 1) * N],
                        start=(c == 0), stop=(c == KC - 1),
                    )

                # row softmax over L
                ngr = small.tile([N, 1], F32, tag="ngr")
                nc.vector.reduce_max(ngr[:], sim_ps[:], axis=AX.X, negate=True)
                er = cq.tile([N, L], F32, tag="er")
                rs = small.tile([N, 1], F32, tag="rs")
                er_i = nc.scalar.activation(
                    er[:], sim_ps[:], AF.Exp, bias=ngr[:], accum_out=rs[:]
                )
                cqa_exp_sink.append(er_i.ins)
                rr = small.tile([N, 1], F32, tag="rr")
                nc.vector.reciprocal(rr[:], rs[:])
                sim_r = cq.tile([N, L], F32, tag="sim_r")
                nc.scalar.activation(sim_r[:], er[:], AF.Copy, scale=rr[:])

                # col softmax over N (free axis of simT)
                ngc = small.tile([L, 1], F32, tag="ngc")
                nc.vector.reduce_max(ngc[:], simT_ps[:], axis=AX.X, negate=True)
                ec = cq.tile([L, N], F32, tag="ec")
                cs = small.tile([L, 1], F32, tag="cs")
                ec_i = nc.scalar.activation(
                    ec[:], simT_ps[:], AF.Exp, bias=ngc[:], accum_out=cs[:]
                )
                cqa_exp_sink.append(ec_i.ins)
                rc = small.tile([L, 1], F32, tag="rc")
                nc.vector.reciprocal(rc[:], cs[:])
                sim_cT = cq.tile([L, N], F32, tag="sim_cT")
                nc.scalar.activation(sim_cT[:], ec[:], AF.Copy, scale=rc[:])

                if gelu_anchor:
                    for gi in gelu_anchor:
                        add_dep_helper(er_i.ins, gi, False, "act-table phase order")
                        add_dep_helper(ec_i.ins, gi, False, "act-table phase order")

                # transposes of the softmaxed maps
                srT_ps = cqp.tile([L, N], F32, tag="cq")
                nc.tensor.matmul(
                    srT_ps[:], sim_r[:], ident[0:N, 0:N], is_transpose=True,
                    start=True, stop=True,
                )
                sim_rT = cq.tile([L, N], F32, tag="sim_rT")
                nc.vector.tensor_copy(sim_rT[:].bitcast(F32R), srT_ps[:])
                sc_ps = cqp.tile([N, L], F32, tag="cq")
                nc.tensor.matmul(
                    sc_ps[:], sim_cT[:], ident[0:L, 0:L], is_transpose=True,
                    start=True, stop=True,
                )
                sim_c = cq.tile([N, L], F32, tag="sim_c")
                nc.vector.tensor_copy(sim_c[:].bitcast(F32R), sc_ps[:])

                # Transposed-direct CQA tail:
                #   A^T[d, n] = sum_l q[l, d] sim_rT[l, n]   (4 small matmuls)
                #   M[l, d]   = sim_c^T @ feat               (1 matmul)
                #   B^T[d, n] = sum_l M[l, d] sim_rT[l, n]   (4 small matmuls)
                #   (feat*A)^T = fT . A^T elementwise, likewise (feat*B)^T —
                # A/B never materialize in [n, d] form, so the 12 PE
                # transposes and 2 full-width matmuls of the naive form go
                # away and the serial chain is shorter.
                at_ps = cqp.tile([128, KC * N], F32, tag="cq")
                for c in range(KC):
                    nc.tensor.matmul(
                        at_ps[:, c * N : (c + 1) * N],
                        _r(q_sb[:, c * 128 : (c + 1) * 128]),
                        _r(sim_rT[:]),
                        start=True, stop=True,
                    )
                AT_sb = cq.tile([128, KC * N], BF16, tag="AT")
                nc.vector.tensor_copy(AT_sb[:], at_ps[:])
                fAT_sb = cq.tile([128, KC * N], BF16, tag="fAT")
                nc.vector.tensor_mul(fAT_sb[:], fT_sb[:], at_ps[:])

                M_ps = cqp.tile([L, D], F32, tag="cq")
                nc.tensor.matmul(
                    M_ps[:], _r(sim_c[:]), _r(feat_ln[:]), start=True, stop=True
                )
                M_sb = cq.tile([L, D], F32, tag="M")
                nc.vector.tensor_copy(M_sb[:].bitcast(F32R), M_ps[:])
                bt_ps = cqp.tile([128, KC * N], F32, tag="cq")
                for c in range(KC):
                    nc.tensor.matmul(
                        bt_ps[:, c * N : (c + 1) * N],
                        _r(M_sb[:, c * 128 : (c + 1) * 128]),
                        _r(sim_rT[:]),
                        start=True, stop=True,
                    )
                fBT_sb = cq.tile([128, KC * N], BF16, tag="fBT")
                nc.vector.tensor_mul(fBT_sb[:], fT_sb[:], bt_ps[:])

                cat_ps = cqp.tile([N, D], F32, tag="cq")
                lhs_list = [fTr_sb, AT_sb, fAT_sb, fBT_sb]
                for c in range(KC):
                    for wi in range(4):
                        nc.tensor.matmul(
                            cat_ps[:],
                            lhs_list[wi][:, c * N : (c + 1) * N],
                            wf_sb[:, wi, c, :],
                            start=(c == 0 and wi == 0),
                            stop=(c == KC - 1 and wi == 3),
                        )
                cat_sb = cq.tile([N, D], F32, tag="cat")
                nc.vector.tensor_add(cat_sb[:], cat_ps[:], bf_bc[:])

                o_sb = cq.tile([N, D], F32, tag="o")
                _layer_norm(
                    nc, cq, small, cat_sb[:], g2_bc[:], be2_bc[:], o_sb, "ln2",
                    eps_t[0:N],
                )
                nc.scalar.dma_start(out_d[b], o_sb[:])

            # ---------------- per batch ----------------
            # Window b: [phase A(b)] -> [CQA(b-1)] -> [phase B(b)].
            # ACT table phases: [gelus(b)] [exps(b) + cqa-exps(b-1)] repeat,
            # enforced with explicit dep edges = 2 table loads per batch.
            prev_exp_anchors = []   # exps(b-1) + cqa exps of b-2
            pending = None          # state of batch b-1 awaiting CQA
            for b in range(BS):
                gelus_b = []
                # ===== phase A (gelu table set): xT -> hT -> selT =====
                selT_full = selnp.tile([N, T], BF16, tag="selT_full")
                x_tiles = []
                q_sb = qp.tile([L, D], F32, tag="q")
                nc.scalar.dma_start(q_sb[:].bitcast(F32R), q_d[b])
                qT_sb = qp.tile([128, KC * L], F32, tag="qT")
                nc.scalar.dma_start(
                    qT_sb[:], qt_d[b].rearrange("p (c l) -> p c l", c=KC)
                )
                for m in range(NMACRO):
                    x_sb = xp.tile([128, JT, D], BF16, tag="x")
                    x_tiles.append(x_sb)
                    nc.gpsimd.dma_start(
                        x_sb[:], x_d[b, m].rearrange("p (j d) -> p j d", j=JT)
                    )
                    xT_sb = xts.tile([128, KC, MACRO], BF16, tag="xT")
                    nc.sync.dma_start(
                        xT_sb[:], xt_d[b, m].rearrange("p (c t) -> p c t", c=KC)
                    )
                    hT_sb = hts.tile([128, KC, MACRO], BF16, tag="hT")
                    for mm in range(KC):
                        h_ps = htp.tile([128, MACRO], F32, tag="h")
                        for k in range(KC):
                            nc.tensor.matmul(
                                h_ps[:],
                                w1_sb[:, k, mm * 128 : (mm + 1) * 128],
                                xT_sb[:, k, :],
                                start=(k == 0),
                                stop=(k == KC - 1),
                            )
                        gi = nc.scalar.activation(
                            hT_sb[:, mm, :], h_ps[:], AF.Gelu,
                            bias=b1T[:, mm : mm + 1],
                        )
                        gelus_b.append(gi.ins)
                    se_ps = sep.tile([128, MACRO], F32, tag="sel")
                    selT = se_ps[0:N, :]
                    for k in range(KC):
                        nc.tensor.matmul(
                            selT,
                            w2_sb[:, k, :],
                            hT_sb[:, k, :],
                            start=(k == 0),
                            stop=(k == KC - 1),
                        )
                    nc.vector.tensor_copy(
                        selT_full[:, m * MACRO : (m + 1) * MACRO], selT
                    )

                # table phase order: gelus after previous window's exps
                for gi in gelus_b:
                    for anchor in prev_exp_anchors:
                        add_dep_helper(gi, anchor, False, "act-table order")

                # ----- deferred CQA of batch b-1 (exps join this window) -----
                cqa_exps = []
                if pending is not None:
                    emit_cqa(pending, cqa_exps, gelus_b)

                # ===== phase B (exp table set) =====
                feat_ps = fpp.tile([N, D], F32, tag="feat")
                dparts = small.tile([N, T // EXPCH], F32, tag="dparts")
                exps_b = []
                for m in range(NMACRO):
                    if m % MPE == 0:
                        E_T = etp.tile([N, EXPCH], F32, tag="E_T")
                        exp_i = nc.scalar.activation(
                            E_T[:],
                            selT_full[:, m * MACRO : m * MACRO + EXPCH],
                            AF.Exp,
                            accum_out=dparts[:, m // MPE : m // MPE + 1],
                        )
                        exps_b.append(exp_i.ins)
                        for gi in gelus_b:
                            add_dep_helper(exp_i.ins, gi, False, "act-table order")
                    eoff = (m % MPE) * MACRO
                    en_ps = cqp.tile([128, JT * N], F32, tag="cq")
                    for j in range(JT):
                        nc.tensor.matmul(
                            en_ps[:, j * N : (j + 1) * N],
                            E_T[:, eoff + j * 128 : eoff + (j + 1) * 128],
                            ident[0:N, 0:N],
                            is_transpose=True,
                            start=True,
                            stop=True,
                        )
                    E_nat = enp.tile([128, JT * N], BF16, tag="E_nat")
                    nc.vector.tensor_copy(E_nat[:], en_ps[:])
                    for j in range(JT):
                        nc.tensor.matmul(
                            feat_ps[:],
                            E_nat[:, j * N : (j + 1) * N],
                            x_tiles[m][:, j, :],
                            start=(m == 0 and j == 0),
                            stop=(m == NMACRO - 1 and j == JT - 1),
                        )

                # eps * denom^2 keeps LN1 exact w/o softmax normalization
                denom = small.tile([N, 1], F32, tag="denom")
                nc.vector.reduce_sum(denom[:], dparts[:], axis=AX.X)
                epsc2 = small.tile([N, 1], F32, tag="epsc2")
                nc.vector.tensor_mul(epsc2[:], denom[:], denom[:])
                nc.vector.tensor_scalar_mul(epsc2[:], epsc2[:], EPS)

                prev_exp_anchors = exps_b + cqa_exps
                pending = {"b": b, "feat_ps": feat_ps, "epsc2": epsc2,
                           "q_sb": q_sb, "qT_sb": qT_sb}

            # final batch's CQA (stays in the last exp window)
            tail_exps = []
            emit_cqa(pending, tail_exps, None)

    nc.compile()
    return nc


_NC_CACHE = None


def kernel(**inputs) -> np.ndarray:
    global _NC_CACHE
    if _NC_CACHE is None:
        _NC_CACHE = build_program()
    nc = _NC_CACHE

    def f32(a):
        return np.ascontiguousarray(np.asarray(a), dtype=np.float32)

    def bf(a):
        return np.asarray(a, dtype=np.float32).astype(ml_dtypes.bfloat16)

    x = bf(inputs["input"])                      # [B, T, D] bf16
    # token-major slabs: [B, NM, 128, JT*D], x_perm[b,m,p] = x[b, m*512+j*128+p, :]
    xp_ = np.ascontiguousarray(
        x.reshape(B, NMACRO, JT, 128, D).transpose(0, 1, 3, 2, 4)
    ).reshape(B, NMACRO, 128, JT * D)
    # feature-major slabs: [B, NM, 128, KC*MACRO],
    # xt_perm[b,m,p] = x[b, m*512:(m+1)*512, c*128+p].T
    xt_ = np.ascontiguousarray(
        x.reshape(B, NMACRO, MACRO, KC, 128).transpose(0, 1, 4, 3, 2)
    ).reshape(B, NMACRO, 128, KC * MACRO)
    q = f32(inputs["query"])
    # qt[b, p, c, l] = q[b, l, c*128+p]
    qt = np.ascontiguousarray(
        q.reshape(B, L, KC, 128).transpose(0, 3, 2, 1)
    ).reshape(B, 128, KC * L)
    w1p = np.ascontiguousarray(
        bf(inputs["w1"]).reshape(KC, 128, D).transpose(1, 0, 2)
    ).reshape(128, KC * D)
    w2p = np.ascontiguousarray(
        bf(inputs["w2"]).reshape(KC, 128, N).transpose(1, 0, 2)
    ).reshape(128, KC * N)
    wfp = np.ascontiguousarray(
        np.stack(
            [bf(inputs[f"wf{i}"]).reshape(KC, 128, D) for i in range(1, 5)]
        ).transpose(2, 0, 1, 3)
    ).reshape(128, 4, KC * D)
    bfsum = (
        f32(inputs["bf1"]) + f32(inputs["bf2"])
        + f32(inputs["bf3"]) + f32(inputs["bf4"])
    )
    shared = {
        "w1": w1p,
        "b1": f32(inputs["b1"]),
        "w2": w2p,
        "g1": f32(inputs["ln1_g"]),
        "be1": f32(inputs["ln1_b"]),
        "wf": wfp,
        "bfsum": bfsum,
        "g2": f32(inputs["ln2_g"]),
        "be2": f32(inputs["ln2_b"]),
    }
    in_maps = []
    for c in range(NCORES):
        m = dict(shared)
        m["x"] = xp_[c * BS : (c + 1) * BS]
        m["xt"] = xt_[c * BS : (c + 1) * BS]
        m["q"] = q[c * BS : (c + 1) * BS]
        m["qt"] = qt[c * BS : (c + 1) * BS]
        in_maps.append(m)

    res = bass_utils.run_bass_kernel_spmd(
        nc, in_maps, core_ids=list(range(NCORES)), trace=TRACE
    )
    if TRACE and res.exec_time_ns is not None:
        print(f"HW exec time: {res.exec_time_ns} ns")
    out = np.concatenate([res.results[c]["out"] for c in range(NCORES)], axis=0)
    return out
